# revision 1
# baseline (speedup 1.0000x reference)
"""SSIM masked loss kernel for Trainium2 (8 NeuronCores, data-parallel over batch).

Strategy
--------
Pure data parallel: B=16 images sharded 2 per core.  Per (b, c) we need the
five 11x11 Gaussian-window maps mu1, mu2, E[x1^2]+E[x2^2], E[x1 x2] plus the
box-filtered validity mask per b.  The separable conv is done entirely on the
TensorEngine as two banded matmuls:

  pass 1 (vertical):   T = X^T @ G1     -- image slice is the *stationary*
                                           operand, so the output lands
                                           transposed ([W, H]) for free
  pass 2 (horizontal): F = G2^T @ T     -- banded Gaussian stationary

H and W are tiled in 118-wide output tiles whose 128-row input windows overlap
by 10 rows, so every output tile is a single K<=128 contraction (no halo
matmuls).  SAME zero-padding is folded into the band matrices (truncated
bands at the image edges).

The per-pixel SSIM arithmetic runs on DVE/ACT/GPSIMD reading the conv results
straight out of PSUM, and the masked reduction is fused into a
tensor_tensor_reduce plus a tiny ones-vector matmul for the cross-partition
sum.  Each core returns per-image partial sums; the final few flops run on
host in numpy.
"""

import sys

import numpy as np

sys.path.insert(0, "/opt/trn_rl_repo")

import concourse.bass as bass  # noqa: E402
import concourse.bacc as bacc  # noqa: E402
from concourse import mybir  # noqa: E402
from concourse.bass_utils import run_bass_kernel_spmd  # noqa: E402
from concourse.tile import TileContext  # noqa: E402

WS = 11
PAD = WS // 2
SIGMA = 1.5
C1 = np.float32(0.01**2)
C2 = np.float32(0.03**2)

B, C, H, W = 16, 3, 512, 512
NCORES = 8
BL = B // NCORES  # images per core

# output tiles of 118 rows; input windows of <=128 rows (clipped at edges)
# (in_start, in_size, out_start, out_size)
TILES = [
    (0, 123, 0, 118),
    (113, 128, 118, 118),
    (231, 128, 236, 118),
    (349, 128, 354, 118),
    (467, 45, 472, 40),
]
NT = len(TILES)

F32 = mybir.dt.float32
AF = mybir.ActivationFunctionType
ALU = mybir.AluOpType


def _gauss_taps() -> np.ndarray:
    x = np.arange(WS, dtype=np.float32) - (WS // 2)
    g = np.exp(-(x**2) / np.float32(2.0 * SIGMA * SIGMA)).astype(np.float32)
    return (g / g.sum()).astype(np.float32)


def _band(in0: int, k: int, o0: int, m: int, taps: np.ndarray) -> np.ndarray:
    """G[i, j] = taps[(in0+i) - (o0+j) + PAD]; zero outside the band."""
    gi = np.arange(k)[:, None] + in0
    gj = np.arange(m)[None, :] + o0
    d = gi - gj + PAD
    ok = (d >= 0) & (d < WS)
    out = np.zeros((k, m), np.float32)
    out[ok] = taps[d[ok]]
    return out


# packed weight layout: 4 variants x 512 cols of band matrices + 1 ones col
_VARS = ["g", "g2x2", "b1", "b2"]
_COL0 = [0, 118, 236, 354, 472]  # column offset of tile j within a variant


def _weight_arrays() -> dict[str, np.ndarray]:
    g = _gauss_taps()
    taps = {
        "g": g,
        "g2x2": (2.0 * g).astype(np.float32),
        "b1": np.ones(WS, np.float32),
        "b2": np.full(WS, np.float32(1.0 / (WS * WS)), np.float32),
    }
    wp = np.zeros((128, 4 * 512 + 1), np.float32)
    for vi, v in enumerate(_VARS):
        for j, (i0, k, o0, m) in enumerate(TILES):
            wp[:k, vi * 512 + _COL0[j] : vi * 512 + _COL0[j] + m] = _band(
                i0, k, o0, m, taps[v]
            )
    wp[:, -1] = 1.0  # ones column for the cross-partition reduction matmul
    return {"wpack": wp}


def _build_bass() -> bass.Bass:
    nc = bacc.Bacc()

    img1_d = nc.declare_dram_parameter("img1", [BL, C, H, W], F32, isOutput=False)
    img2_d = nc.declare_dram_parameter("img2", [BL, C, H, W], F32, isOutput=False)
    match_d = nc.declare_dram_parameter("matchf", [BL, 1, H, W], F32, isOutput=False)
    wpack_d = nc.declare_dram_parameter("wpack", [128, 4 * 512 + 1], F32, isOutput=False)
    out_d = nc.declare_dram_parameter("out", [BL, 2], F32, isOutput=True)

    from contextlib import ExitStack

    with TileContext(nc) as tc, ExitStack() as ctx:
        consts = ctx.enter_context(tc.tile_pool(name="consts", bufs=1))
        imgs = ctx.enter_context(tc.tile_pool(name="imgs", bufs=2))
        pre = ctx.enter_context(tc.tile_pool(name="pre", bufs=2))
        tsbp = ctx.enter_context(tc.tile_pool(name="tsb", bufs=2))
        pix = ctx.enter_context(tc.tile_pool(name="pix", bufs=2))
        maskp = ctx.enter_context(tc.tile_pool(name="maskp", bufs=1))
        accp = ctx.enter_context(tc.tile_pool(name="accp", bufs=2))
        outp = ctx.enter_context(tc.tile_pool(name="outp", bufs=2))
        psumT = ctx.enter_context(tc.tile_pool(name="psumT", bufs=4, space="PSUM"))
        psumF = ctx.enter_context(tc.tile_pool(name="psumF", bufs=3, space="PSUM"))
        psumR = ctx.enter_context(tc.tile_pool(name="psumR", bufs=1, space="PSUM"))

        # stage all band matrices in SBUF with one DMA
        wpack = consts.tile([128, 4 * 512 + 1], F32, tag="wpack", name="wpack")
        nc.sync.dma_start(out=wpack, in_=wpack_d[:, :])

        def wsl(var: str, j: int):
            vi = _VARS.index(var)
            i0, k, o0, m = TILES[j]
            c0 = vi * 512 + _COL0[j]
            return wpack[:k, c0 : c0 + m]

        def conv_pass1(src_tiles, var, wb):
            """vertical conv + transpose: returns PSUM tile [kw, 512] where
            kw is the width of W-window wb; column range j holds out-rows."""
            w0, kw, _, _ = TILES[wb]
            tp = psumT.tile([128, 512], F32, tag="T")
            for j, (i0, k, o0, m) in enumerate(TILES):
                nc.tensor.matmul(
                    tp[:kw, o0 : o0 + m],
                    src_tiles[j][:k, w0 : w0 + kw],
                    wsl(var, j),
                    start=True,
                    stop=True,
                )
            return tp

        def evac(idx, dst, src):
            if idx % 2 == 0:
                nc.vector.tensor_copy(dst, src)
            else:
                nc.scalar.copy(dst, src)

        for b in range(BL):
            # ---------------- mask pipeline (box conv of match) -------------
            mt = [imgs.tile([128, 512], F32, tag=f"match_{j}", name=f"match_{j}") for j in range(NT)]
            for j, (i0, k, o0, m) in enumerate(TILES):
                nc.sync.dma_start(out=mt[j][:k, :], in_=match_d[b, 0, i0 : i0 + k, :])

            mask_sb = []
            mcols = accp.tile([128, NT], F32, tag="mcols")
            nc.vector.memset(mcols, 0.0)
            for wb, (w0, kw, ow0, mw) in enumerate(TILES):
                tp = conv_pass1(mt, "b1", wb)
                tsb = tsbp.tile([128, 512], F32, tag="tsb_m")
                evac(wb, tsb[:kw, :], tp[:kw, :])
                fp = psumF.tile([128, 512], F32, tag="F")
                nc.tensor.matmul(
                    fp[:mw, :], wsl("b2", wb), tsb[:kw, :],
                    start=True, stop=True,
                )
                mk = maskp.tile([128, 512], F32, tag=f"mask_{wb}")
                nc.vector.tensor_scalar(
                    mk[:mw, :], fp[:mw, :], 0.5, 1e-7,
                    ALU.is_gt, ALU.add,
                    accum_out=mcols[:mw, wb : wb + 1],
                )
                mask_sb.append(mk)

            scols = accp.tile([128, C * NT], F32, tag="scols")
            nc.vector.memset(scols, 0.0)

            # ---------------- channels -------------------------------------
            for c in range(C):
                x1 = [imgs.tile([128, 512], F32, tag=f"x1_{j}", name=f"x1_{j}") for j in range(NT)]
                x2 = [imgs.tile([128, 512], F32, tag=f"x2_{j}", name=f"x2_{j}") for j in range(NT)]
                for j, (i0, k, o0, m) in enumerate(TILES):
                    nc.sync.dma_start(out=x1[j][:k, :], in_=img1_d[b, c, i0 : i0 + k, :])
                    nc.sync.dma_start(out=x2[j][:k, :], in_=img2_d[b, c, i0 : i0 + k, :])

                p12 = [pre.tile([128, 512], F32, tag=f"p12_{j}", name=f"p12_{j}") for j in range(NT)]
                ssq = [pre.tile([128, 512], F32, tag=f"ssq_{j}", name=f"ssq_{j}") for j in range(NT)]
                for j, (i0, k, o0, m) in enumerate(TILES):
                    s1 = pre.tile([128, 512], F32, tag="sq_a")
                    s2 = pre.tile([128, 512], F32, tag="sq_b")
                    nc.scalar.square(s1[:k, :], x1[j][:k, :])
                    nc.scalar.square(s2[:k, :], x2[j][:k, :])
                    nc.vector.tensor_mul(p12[j][:k, :], x1[j][:k, :], x2[j][:k, :])
                    nc.gpsimd.tensor_add(ssq[j][:k, :], s1[:k, :], s2[:k, :])

                for wb, (w0, kw, ow0, mw) in enumerate(TILES):
                    srcs = (x1, x2, p12, ssq)
                    tsbs = []
                    for mi, src in enumerate(srcs):
                        tp = conv_pass1(src, "g", wb)
                        tsb = tsbp.tile([128, 512], F32, tag=f"tsb_{mi}")
                        evac(mi, tsb[:kw, :], tp[:kw, :])
                        tsbs.append(tsb)
                    fm1 = psumF.tile([128, 512], F32, tag="F")
                    fm2 = psumF.tile([128, 512], F32, tag="F")
                    fr2 = psumF.tile([128, 512], F32, tag="F")
                    fs = psumF.tile([128, 512], F32, tag="F")
                    nc.tensor.matmul(fm1[:mw, :], wsl("g", wb),
                                     tsbs[0][:kw, :], start=True, stop=True)
                    nc.tensor.matmul(fm2[:mw, :], wsl("g", wb),
                                     tsbs[1][:kw, :], start=True, stop=True)
                    nc.tensor.matmul(fr2[:mw, :], wsl("g2x2", wb),
                                     tsbs[2][:kw, :], start=True, stop=True)
                    nc.tensor.matmul(fs[:mw, :], wsl("g", wb),
                                     tsbs[3][:kw, :], start=True, stop=True)

                    # ---- per-pixel SSIM ------------------------------------
                    m1 = pix.tile([128, 512], F32, tag="m1")
                    m1s = pix.tile([128, 512], F32, tag="m1s")
                    m2s = pix.tile([128, 512], F32, tag="m2s")
                    nc.scalar.copy(m1[:mw, :], fm1[:mw, :])
                    nc.scalar.square(m1s[:mw, :], fm1[:mw, :])
                    nc.scalar.square(m2s[:mw, :], fm2[:mw, :])

                    pm = pix.tile([128, 512], F32, tag="pm")
                    nc.vector.tensor_mul(pm[:mw, :], m1[:mw, :], fm2[:mw, :])
                    num1 = pix.tile([128, 512], F32, tag="num1")
                    nc.vector.tensor_scalar(
                        num1[:mw, :], pm[:mw, :], 2.0, float(C1), ALU.mult, ALU.add
                    )
                    q = pix.tile([128, 512], F32, tag="q")
                    nc.gpsimd.tensor_add(q[:mw, :], m1s[:mw, :], m2s[:mw, :])
                    den1 = pix.tile([128, 512], F32, tag="den1")
                    nc.vector.tensor_scalar_add(den1[:mw, :], q[:mw, :], float(C1))
                    num2 = pix.tile([128, 512], F32, tag="num2")
                    nc.vector.affine_then_add(
                        num2[:mw, :], num1[:mw, :], fr2[:mw, :],
                        scale=-1.0, bias=float(C1 + C2),
                    )
                    den2 = pix.tile([128, 512], F32, tag="den2")
                    nc.vector.affine_then_add(
                        den2[:mw, :], q[:mw, :], fs[:mw, :],
                        scale=-1.0, bias=float(C2),
                    )
                    num = pix.tile([128, 512], F32, tag="num")
                    nc.vector.tensor_mul(num[:mw, :], num1[:mw, :], num2[:mw, :])
                    den = pix.tile([128, 512], F32, tag="den")
                    nc.vector.tensor_mul(den[:mw, :], den1[:mw, :], den2[:mw, :])
                    rec = pix.tile([128, 512], F32, tag="rec")
                    nc.vector.reciprocal_approx_fast(out=rec[:mw, :], in_=den[:mw, :])
                    s = pix.tile([128, 512], F32, tag="s")
                    nc.vector.tensor_mul(s[:mw, :], num[:mw, :], rec[:mw, :])
                    junk = pix.tile([128, 512], F32, tag="pm", name="junk")
                    nc.vector.tensor_mul(junk[:mw, :], s[:mw, :], mask_sb[wb][:mw, :])
                    nc.vector.tensor_reduce(
                        scols[:mw, c * NT + wb : c * NT + wb + 1], junk[:mw, :],
                        mybir.AxisListType.X, ALU.add,
                    )

            # ---------------- per-image reduction --------------------------
            fin = accp.tile([128, 2], F32, tag="fin")
            nc.vector.tensor_reduce(fin[:, 0:1], scols[:, :], mybir.AxisListType.X, ALU.add)
            nc.vector.tensor_reduce(fin[:, 1:2], mcols[:, :], mybir.AxisListType.X, ALU.add)
            fr = psumR.tile([2, 1], F32, tag="fin_ps")
            nc.tensor.matmul(fr[:2, :1], fin[:, :2], wpack[:, -1:],
                             start=True, stop=True)
            osb = outp.tile([2, 1], F32, tag="osb")
            nc.scalar.copy(osb[:2, :1], fr[:2, :1])
            nc.sync.dma_start(out=out_d[b, :], in_=osb[:2, 0:1])

    nc.compile()
    return nc


_NC_CACHE: bass.Bass | None = None


def _get_nc() -> bass.Bass:
    global _NC_CACHE
    if _NC_CACHE is None:
        _NC_CACHE = _build_bass()
    return _NC_CACHE


def kernel(img1: np.ndarray, img2: np.ndarray, match: np.ndarray) -> np.ndarray:
    img1 = np.ascontiguousarray(img1, dtype=np.float32)
    img2 = np.ascontiguousarray(img2, dtype=np.float32)
    matchf = np.ascontiguousarray(match.astype(np.float32))

    weights = _weight_arrays()
    in_maps = []
    for i in range(NCORES):
        sl = slice(i * BL, (i + 1) * BL)
        m = {"img1": img1[sl], "img2": img2[sl], "matchf": matchf[sl]}
        m.update(weights)
        in_maps.append(m)

    nc = _get_nc()
    res = run_bass_kernel_spmd(nc, in_maps, list(range(NCORES))).results

    total = np.float64(0.0)
    for i in range(NCORES):
        o = np.asarray(res[i]["out"], dtype=np.float64)  # [BL, 2]
        s1 = o[:, 0]
        mb = o[:, 1]
        s_b = 3.0 * mb - s1
        norm = (H * W) / (mb + 1e-6)
        total += np.sum(s_b * norm)
    return np.float32(total / 3.0)

